# revision 20
# baseline (speedup 1.0000x reference)
"""RBF/KNN interpolation kernel for Trainium2 (8 NeuronCores, data parallel).

Computes, per batch b:
    v        = input_data[b, -1, :, 0]                      (N_in,)
    w[o, i]  = exp(-||tc[o] - ic[i]||^2 / (2 * 0.1^2))      (N_out, N_in)
    interp   = (w @ v) / (w.sum(-1) + 1e-8)                 (N_out,)
    out[b]   = broadcast(interp) -> (n_samples, N_out, 4)

Sharding: batch B=8 across 8 cores (one batch per core).

v2 strategy on top of the v1 on-chip weight-matrix build:
  - Host sorts input points and target points by x per batch (a pure
    permutation; inverse-applied to the output). With sorted points,
    i-chunks (128 inputs) and o-subtiles (512 targets) cover narrow
    x-quantile windows, so (i-chunk, o-sub) pairs whose nominal windows
    are farther apart than D_WINDOW are statically pruned: their RBF
    weights are < exp(-50 * 0.3^2) and contribute ~0 to num/den.
  - The Exp over the kept logit tiles is split across engines: the
    Scalar engine runs true Exp; the Vector engine runs a Schraudolph
    fp16 bit-trick exp (y = A*logit + B -> int16 -> bitcast fp16), with
    the clamp/convert second pass placed on Vector or GpSimd by a
    compile-time greedy load balancer.
  - The PE stream is software-pipelined: the num/den matmul of tile t
    is emitted after the logits matmul of tile t+1 so the in-order PE
    never stalls waiting for an exp.
"""

from contextlib import ExitStack
from functools import lru_cache

import numpy as np

import concourse.bass as bass
import concourse.bacc as bacc
import concourse.tile as tile
from concourse import mybir
from concourse.bass_utils import run_bass_kernel_spmd

F32 = mybir.dt.float32
F16 = mybir.dt.float16
I16 = mybir.dt.int16
AF = mybir.ActivationFunctionType
ALU = mybir.AluOpType

# Problem sizes (hardcoded per spec)
B = 8
T_IN = 4
N_IN = 4096
V_IN = 3
N_OUT = 8192
S = 10
T_OUT = 4
GAMMA = 50.0  # 1 / (2 * LENGTH_SCALE^2), LENGTH_SCALE = 0.1
EPS = 1e-8
WSCALE_LOG = 6.93147180559945  # ln(2^10)
WSCALE = 1024.0

# pruning: drop (i-chunk, o-sub) pairs whose nominal x-quantile windows are
# farther apart than D_WINDOW (includes sort-quantile slack ~0.035)
D_WINDOW = 0.39
IC = N_IN // 128      # 32 i-chunks
OSUB_W = 512
OSUB = N_OUT // OSUB_W  # 16 o-subtiles

# Schraudolph fast-exp constants. Working in t-units (t = y/1024 = fp16
# exponent+15): t = SCH_SCALE*psum + (A_LN*bias + SCH_B)/1024; the stored
# bits are i16 = round(1024*t) clamped at 0 and the mantissa-linear error
# is corrected by w *= 1 + SCH_C*(s^2 - |s|), s = round(t) - t.
A_LN = 1024.0 * 1.4426950408889634           # 1024*log2(e)
SCH_SCALE = 2.0 * GAMMA * A_LN / 1024.0
SCH_B = 15.0                                  # fp16 exponent bias, t-units
EXP_SPLIT = True
SCH_C = 0.2355                                # quadratic mantissa correction coef
PIPE_DELAY = 5                                # tiles between logits emit and nd emit

# per-pass engine cost estimates (ns) for the compile-time balancer
def _t_scalar(cols):  # true exp on Activation engine
    return 0.833 * cols + 216.0
def _t_f32(cols):     # f32 pass on DVE (p1 psum, p2 conv, p3 magic, p4 s)
    return 1.0417 * cols + 90.0
def _t_fast(cols):    # f16 passes p5..p8, assumed 2x
    return 0.55 * cols + 80.0
def _t_gp(cols):      # a pass on GpSimd (eff ~0.6)
    return 1.39 * cols + 100.0


def _plan_tiles():
    """Static pruning plan. Returns (tiles, ic_lo, ic_hi) where tiles is the
    ordered list of (oc, ici, subs, c0, c1) and subs is the kept o-subs."""
    kept = {}
    for i in range(IC):
        for j in range(OSUB):
            ilo, ihi = i / IC, (i + 1) / IC
            olo, ohi = j / OSUB, (j + 1) / OSUB
            gap = max(olo - ihi, ilo - ohi, 0.0)
            if gap < D_WINDOW:
                kept.setdefault(j, []).append(i)
    ic_lo = {j: min(v) for j, v in kept.items()}
    ic_hi = {j: max(v) for j, v in kept.items()}
    tiles = []
    OC = N_OUT // 1024
    for oc in range(OC):
        ja, jb = 2 * oc, 2 * oc + 1
        lo = min(ic_lo[ja], ic_lo[jb])
        hi = max(ic_hi[ja], ic_hi[jb])
        for ici in range(lo, hi + 1):
            subs = [j for j in (ja, jb) if ic_lo[j] <= ici <= ic_hi[j]]
            if not subs:
                continue
            c0 = (min(subs) - ja) * OSUB_W
            c1 = (max(subs) + 1 - ja) * OSUB_W
            tiles.append((oc, ici, subs, c0, c1))
    return tiles, ic_lo, ic_hi


def _assign_engines(tiles):
    """Greedy balance across Scalar/DVE/GpSimd.

    Modes: 'S' scalar true exp; 'V' full Schraudolph+correction on DVE;
    'VG' DVE runs p1/p3/p4/p5, GpSimd runs p2/p6; 'VG2' GpSimd runs p2 only.
    """
    if not EXP_SPLIT:
        return ["S"] * len(tiles)
    load = {"S": 2500.0, "V": 14000.0, "G": 16000.0}
    out = []
    for (_, _, _, c0, c1) in tiles:
        cols = c1 - c0
        cand = []
        for mode in ("S", "V", "VG", "VG2"):
            trial = dict(load)
            if mode == "S":
                trial["S"] += _t_scalar(cols)
            elif mode == "V":
                trial["V"] += 4 * _t_f32(cols) + 4 * _t_fast(cols)
            elif mode == "VG":
                # GP takes p2 (conv) and p8 (final multiply)
                trial["V"] += 3 * _t_f32(cols) + 3 * _t_fast(cols)
                trial["G"] += 2 * _t_gp(cols)
            else:
                # GP takes p2 only
                trial["V"] += 3 * _t_f32(cols) + 4 * _t_fast(cols)
                trial["G"] += _t_gp(cols)
            cand.append((max(trial.values()), sum(trial.values()), mode, trial))
        cand.sort(key=lambda x: (x[0], x[1]))
        _, _, mode, trial = cand[0]
        load = trial
        out.append(mode)
    return out


def build_kernel(tc_ctx, v_h, ic_h, tc_h, out_h, n_in, n_out, s, F=1024):
    tcx = tc_ctx
    nc = tcx.nc
    OC = n_out // F    # o-chunks
    L = n_out // 128   # per-partition interp count in output layout
    CT = n_out // 128  # nat-layout columns (target side)

    tiles, ic_lo, ic_hi = _plan_tiles()
    modes = _assign_engines(tiles)
    # last tile index per oc (for finalize emission)
    last_of_oc = {}
    for idx, (oc, *_rest) in enumerate(tiles):
        last_of_oc[oc] = idx

    with ExitStack() as ctx:
        const_pool = ctx.enter_context(tcx.tile_pool(name="const", bufs=1))

        # ---- persistent tiles ----
        tc_aug = const_pool.tile([128, n_out], F16)  # rows t2h t2l txh txl txh tyh tyl tyh
        ic_aug = const_pool.tile([128, n_in], F16)   # rows 1   1   xh  xh  xl  yh  yh  yl
        bias_nat = const_pool.tile([128, IC], F32)
        ybias_nat = const_pool.tile([128, IC], F32)  # A_LN*bias + SCH_B
        vo_nat = const_pool.tile([128, 33 * IC], F16)  # [v, 0 x31, 1] per chunk
        ident = const_pool.tile([128, 128], F16)

        # ---- head: inputs, identity, splits ----
        head = ctx.enter_context(tcx.tile_pool(name="head", bufs=1))
        tcx_nat = head.tile([128, CT], F32)
        tcy_nat = head.tile([128, CT], F32)
        icx_nat = head.tile([128, IC], F32)
        icy_nat = head.tile([128, IC], F32)
        v_nat = head.tile([128, IC], F32)

        # PE clock warm-up (HAM un-throttles after ~3.4us of full-array work)
        warm_cm = tcx.tile_pool(name="warm_ps", bufs=2, space="PSUM")
        warm_ps = warm_cm.__enter__()
        wsrc = head.tile([128, 512], F16)
        nc.gpsimd.memset(wsrc[:, :].bitcast(mybir.dt.uint32), 0)

        tc_aug_u = tc_aug[:, :].bitcast(mybir.dt.uint32)
        ic_aug_u = ic_aug[:, :].bitcast(mybir.dt.uint32)
        nc.vector.memset(tc_aug_u[:, :tc_aug_u.shape[1] // 2], 0)
        nc.gpsimd.memset(tc_aug_u[:, tc_aug_u.shape[1] // 2:], 0)
        nc.vector.memset(ic_aug_u, 0)

        tc_r = tc_h[:].rearrange("(c p) d -> p c d", p=128)
        ic_r = ic_h[:].rearrange("(c p) d -> p c d", p=128)
        h = CT // 2
        nc.sync.dma_start(out=tcx_nat[:, 0:h].rearrange("p (c o) -> p c o", o=1),
                          in_=tc_r[:, 0:h, 0:1])
        nc.gpsimd.dma_start(out=tcx_nat[:, h:].rearrange("p (c o) -> p c o", o=1),
                            in_=tc_r[:, h:, 0:1])
        nc.scalar.dma_start(out=tcy_nat[:, 0:h].rearrange("p (c o) -> p c o", o=1),
                            in_=tc_r[:, 0:h, 1:2])
        nc.sync.dma_start(out=tcy_nat[:, h:].rearrange("p (c o) -> p c o", o=1),
                          in_=tc_r[:, h:, 1:2])
        nc.gpsimd.dma_start(out=icx_nat.rearrange("p (c o) -> p c o", o=1),
                            in_=ic_r[:, :, 0:1])
        nc.scalar.dma_start(out=icy_nat.rearrange("p (c o) -> p c o", o=1),
                            in_=ic_r[:, :, 1:2])
        nc.sync.dma_start(out=v_nat[:, :],
                          in_=v_h[:].rearrange("(c p) -> p c", p=128))

        # identity for PE transposes
        jj = head.tile([128, 128], F32)
        kk = head.tile([128, 1], F32)
        nc.gpsimd.iota(jj[:, :], [[1, 128]], base=0, channel_multiplier=0,
                       allow_small_or_imprecise_dtypes=True)
        nc.gpsimd.iota(kk[:, :], [[0, 1]], base=0, channel_multiplier=1,
                       allow_small_or_imprecise_dtypes=True)
        nc.gpsimd.tensor_scalar(ident[:, :], jj[:, :], kk[:, 0:1], None,
                                op0=ALU.is_equal)

        # --- target-side nat computes ---
        sqx_t = head.tile([128, CT], F32)
        t2s = head.tile([128, CT], F32)
        nc.gpsimd.tensor_mul(sqx_t[:, :], tcx_nat[:, :], tcx_nat[:, :])
        nc.vector.scalar_tensor_tensor(t2s[:, :], tcy_nat[:, :], 1.0,
                                       tcy_nat[:, :], op0=ALU.bypass,
                                       op1=ALU.mult)
        nc.vector.tensor_add(t2s[:, :], t2s[:, :], sqx_t[:, :])

        t2h_nat = head.tile([128, CT], F16)
        t2l_nat = head.tile([128, CT], F16)
        nc.vector.tensor_scalar_mul(t2h_nat[:, :], t2s[:, :], -0.5)
        nc.vector.scalar_tensor_tensor(t2l_nat[:, :], t2s[:, :], -0.5,
                                       t2h_nat[:, :], op0=ALU.mult,
                                       op1=ALU.subtract)
        txh_nat = head.tile([128, CT], F16)
        txl_nat = head.tile([128, CT], F16)
        tyh_nat = head.tile([128, CT], F16)
        tyl_nat = head.tile([128, CT], F16)
        nc.gpsimd.tensor_copy(txh_nat[:, :], tcx_nat[:, :])
        nc.vector.tensor_sub(txl_nat[:, :], tcx_nat[:, :], txh_nat[:, :])
        nc.gpsimd.tensor_copy(tyh_nat[:, :], tcy_nat[:, :])
        nc.vector.tensor_sub(tyl_nat[:, :], tcy_nat[:, :], tyh_nat[:, :])

        # --- input-side nat computes ---
        sqx_i = head.tile([128, IC], F32)
        i2s = head.tile([128, IC], F32)
        nc.gpsimd.tensor_mul(sqx_i[:, :], icx_nat[:, :], icx_nat[:, :])
        nc.vector.scalar_tensor_tensor(i2s[:, :], icy_nat[:, :], 1.0,
                                       icy_nat[:, :], op0=ALU.bypass,
                                       op1=ALU.mult)
        nc.vector.tensor_add(i2s[:, :], i2s[:, :], sqx_i[:, :])
        # bias = -50 * i2 + ln(2^10);  ybias = A_LN * bias + SCH_B
        nc.vector.tensor_scalar(bias_nat[:, :], i2s[:, :], -GAMMA, WSCALE_LOG,
                                op0=ALU.mult, op1=ALU.add)
        nc.vector.tensor_scalar(ybias_nat[:, :], bias_nat[:, :], A_LN / 1024.0,
                                SCH_B, op0=ALU.mult, op1=ALU.add)

        xh_nat = head.tile([128, IC], F16)
        xl_nat = head.tile([128, IC], F16)
        yh_nat = head.tile([128, IC], F16)
        yl_nat = head.tile([128, IC], F16)
        nc.gpsimd.tensor_copy(xh_nat[:, :], icx_nat[:, :])
        nc.vector.tensor_sub(xl_nat[:, :], icx_nat[:, :], xh_nat[:, :])
        nc.gpsimd.tensor_copy(yh_nat[:, :], icy_nat[:, :])
        nc.vector.tensor_sub(yl_nat[:, :], icy_nat[:, :], yh_nat[:, :])

        # vo_nat: col 33c = v (fp16), col 33c+32 = 1.0, rest 0
        nc.gpsimd.memset(vo_nat[:, :].bitcast(mybir.dt.uint16), 0)
        vo3 = vo_nat.rearrange("p (c w) -> p c w", w=33)
        nc.vector.tensor_copy(vo3[:, :, 0], v_nat[:, :])
        nc.vector.memset(vo3[:, :, 32], 1.0)

        # --- nat -> row layout via PE transpose + copy + DMA ---
        with tcx.tile_pool(name="tps", bufs=2, space="PSUM") as tp_pool, \
             tcx.tile_pool(name="tsb", bufs=2) as tsb_pool:

            tp_count = [0]

            dma_engs = [nc.sync, nc.scalar, nc.gpsimd]

            def to_rows(nat, ncols, aug, rows):
                ps = tp_pool.tile([128, 128], F16, tag="ps")
                sb = tsb_pool.tile([128, 128], F16, tag="sb")
                nc.tensor.transpose(ps[:ncols, :], nat[:, :], ident[:, :])
                if tp_count[0] % 2 == 0:
                    nc.vector.tensor_copy(sb[:ncols, :], ps[:ncols, :])
                else:
                    nc.scalar.copy(sb[:ncols, :], ps[:ncols, :])
                for r in rows:
                    dma_engs[tp_count[0] % 3].dma_start(
                        out=aug[r:r + 1, :].rearrange("r (c p) -> r c p", p=128),
                        in_=sb[:ncols, :],
                    )
                    tp_count[0] += 1

            to_rows(t2h_nat, CT, tc_aug, [0])
            to_rows(t2l_nat, CT, tc_aug, [1])
            to_rows(txh_nat, CT, tc_aug, [2, 4])
            to_rows(txl_nat, CT, tc_aug, [3])
            to_rows(tyh_nat, CT, tc_aug, [5, 7])
            to_rows(tyl_nat, CT, tc_aug, [6])

            nc.vector.memset(ic_aug[0:2, :].bitcast(mybir.dt.uint32), 0x3C003C00)
            to_rows(xh_nat, IC, ic_aug, [2, 3])
            to_rows(xl_nat, IC, ic_aug, [4])
            to_rows(yh_nat, IC, ic_aug, [5, 6])
            to_rows(yl_nat, IC, ic_aug, [7])
            # HAM warm-up bridge: ~5us of sustained full-array matmuls right
            # before the main loop un-throttles the PE clock (1.2 -> 2.4 GHz)
            for _ in range(24):
                wps = warm_ps.tile([128, 512], F32, tag="warm")
                nc.tensor.matmul(wps[:, :], wsrc[:, 0:128], wsrc[:, :],
                                 start=True, stop=True)

        warm_cm.__exit__(None, None, None)

        # ---- main loop (software-pipelined) ----
        PG2 = F // L  # output partitions finalized per o-chunk

        with (
            tcx.tile_pool(name="psum_l", bufs=3, space="PSUM") as pl_pool,
            tcx.tile_pool(name="psum_nd", bufs=1, space="PSUM") as nd_pool,
            tcx.tile_pool(name="w", bufs=8) as w_pool,
            tcx.tile_pool(name="y", bufs=4) as y_pool,
            tcx.tile_pool(name="sch", bufs=3) as sch_pool,
            tcx.tile_pool(name="grp", bufs=2) as grp_pool,
        ):
            # one persistent psum tile; even oc accumulates at partitions
            # 0..32, odd oc at 64..96 (num row +0, den row +32)
            nd = nd_pool.tile([128, F], F32, tag="nd", name="nd")
            pending = []  # (tile_idx, oc, ici, subs, w_tile)

            def emit_front(idx):
                oc, ici, subs, c0, c1 = tiles[idx]
                mode = modes[idx]
                pl = pl_pool.tile([128, F], F32, tag="pl")
                for j in subs:
                    s0 = (j - 2 * oc) * OSUB_W
                    nc.tensor.matmul(
                        pl[:, s0:s0 + OSUB_W],
                        ic_aug[:, ici * 128:(ici + 1) * 128],
                        tc_aug[:, j * OSUB_W:(j + 1) * OSUB_W],
                        start=True, stop=True,
                    )
                w = w_pool.tile([128, F], F16, tag="w")
                if mode == "S":
                    nc.scalar.activation(
                        w[:, c0:c1], pl[:, c0:c1], AF.Exp,
                        bias=bias_nat[:, ici:ici + 1], scale=2.0 * GAMMA,
                    )
                else:
                    # p1: t = fp16-exponent+15 of the weight, in t-units
                    t = y_pool.tile([128, F], F32, tag="t")
                    nc.vector.tensor_scalar(
                        t[:, c0:c1], pl[:, c0:c1],
                        SCH_SCALE, ybias_nat[:, ici:ici + 1],
                        op0=ALU.mult, op1=ALU.add,
                    )
                    # p2: stored bits = round(1024*t) clamped at 0
                    wi = w[:, c0:c1].bitcast(I16)
                    eng2 = nc.gpsimd if mode in ("VG", "VG2") else nc.vector
                    eng2.tensor_scalar(wi, t[:, c0:c1], 1024.0, 0.0,
                                       op0=ALU.mult, op1=ALU.max)
                    # p3/p4: s = round(t) - t via the fp32 2^23 magic trick
                    z = y_pool.tile([128, F], F32, tag="z")
                    ss = sch_pool.tile([128, F], F16, tag="ss")
                    u = sch_pool.tile([128, F], F16, tag="u")
                    hh = sch_pool.tile([128, F], F16, tag="hh")
                    nc.vector.tensor_scalar(
                        z[:, c0:c1], t[:, c0:c1], 2.0 ** 23, None, op0=ALU.add)
                    nc.vector.scalar_tensor_tensor(
                        ss[:, c0:c1], z[:, c0:c1], -2.0 ** 23, t[:, c0:c1],
                        op0=ALU.add, op1=ALU.subtract,
                    )
                    # p5..p8: w *= 1 + SCH_C*(|s|^2 - |s|)  (== f^2 - f)
                    nc.vector.scalar_tensor_tensor(
                        u[:, c0:c1], ss[:, c0:c1], -1.0, ss[:, c0:c1],
                        op0=ALU.mult, op1=ALU.max,
                    )
                    nc.vector.scalar_tensor_tensor(
                        u[:, c0:c1], u[:, c0:c1], -1.0, u[:, c0:c1],
                        op0=ALU.add, op1=ALU.mult,
                    )
                    nc.vector.tensor_scalar(
                        hh[:, c0:c1], u[:, c0:c1], SCH_C, 1.0,
                        op0=ALU.mult, op1=ALU.add,
                    )
                    eng8 = nc.gpsimd if mode == "VG" else nc.vector
                    eng8.tensor_mul(w[:, c0:c1], w[:, c0:c1], hh[:, c0:c1])
                pending.append((idx, oc, ici, subs, w))

            def emit_back():
                idx, oc, ici, subs, w = pending.pop(0)
                poff = 64 * (oc % 2)
                for j in subs:
                    s0 = (j - 2 * oc) * OSUB_W
                    nc.tensor.matmul(
                        nd[poff:poff + 33, s0:s0 + OSUB_W],
                        vo_nat[:, 33 * ici:33 * ici + 33],
                        w[:, s0:s0 + OSUB_W],
                        start=(ici == ic_lo[j]), stop=(ici == ic_hi[j]),
                        skip_group_check=True,
                    )
                if idx == last_of_oc[oc]:
                    finalize(oc)

            def finalize(oc):
                poff = 64 * (oc % 2)
                nd_rows = grp_pool.tile([33, F], F32, tag="ndrows")
                nc.vector.tensor_copy(nd_rows[0:33, :], nd[poff:poff + 33, :])
                gnum = grp_pool.tile([PG2, L], F32, tag="gnum")
                gden = grp_pool.tile([PG2, L], F32, tag="gden")
                grep = grp_pool.tile([PG2, 4 * L], F32, tag="grep")
                nc.sync.dma_start(
                    out=gnum[:, :],
                    in_=nd_rows[0:1, :].rearrange("r (p k) -> r p k", k=L),
                )
                nc.gpsimd.dma_start(
                    out=gden[:, :],
                    in_=nd_rows[32:33, :].rearrange("r (p k) -> r p k", k=L),
                )
                nc.vector.tensor_scalar_add(gden[:, :], gden[:, :], EPS * WSCALE)
                nc.vector.reciprocal(gden[:, :], gden[:, :])
                nc.vector.tensor_mul(gnum[:, :], gnum[:, :], gden[:, :])
                grep3 = grep.rearrange("p (k t) -> p k t", t=4)
                for t in range(4):
                    nc.vector.tensor_copy(grep3[:, :, t], gnum[:, :])
                engs = [nc.sync, nc.gpsimd]
                for si in range(s):
                    engs[si % len(engs)].dma_start(
                        out=out_h[:][si].rearrange("o t -> (o t)").rearrange(
                            "(p j) -> p j", p=n_out * 4 // (4 * L))[
                                oc * PG2:(oc + 1) * PG2, :],
                        in_=grep[:, :],
                    )

            # software pipeline: nd matmul of tile t lands PIPE_DELAY after
            # its logits matmul, so the in-order PE never waits on an exp
            for idx in range(len(tiles)):
                emit_front(idx)
                if len(pending) > PIPE_DELAY:
                    emit_back()
            while pending:
                emit_back()


@lru_cache(maxsize=2)
def build_nc(n_in=N_IN, n_out=N_OUT, s=S, F=1024):
    nc = bacc.Bacc("TRN2", target_bir_lowering=False, debug=False)
    v_h = nc.dram_tensor("v", [n_in], F32, kind="ExternalInput")
    ic_h = nc.dram_tensor("ic", [n_in, 2], F32, kind="ExternalInput")
    tc_h = nc.dram_tensor("tc", [n_out, 2], F32, kind="ExternalInput")
    out_h = nc.dram_tensor("out", [s, n_out, T_OUT], F32, kind="ExternalOutput")
    with tile.TileContext(nc) as tcx:
        build_kernel(tcx, v_h, ic_h, tc_h, out_h, n_in, n_out, s, F=F)
    nc.compile()
    return nc


def _run(input_data, input_coords, target_coords, n_samples, trace=False):
    n_samples = int(n_samples)
    assert n_samples == S, f"kernel compiled for n_samples={S}, got {n_samples}"
    assert input_data.shape == (B, T_IN, N_IN, V_IN)
    nc = build_nc()
    perms_o = []
    in_maps = []
    for b in range(B):
        perm_i = np.argsort(input_coords[b, :, 0], kind="stable")
        perm_o = np.argsort(target_coords[b, :, 0], kind="stable")
        perms_o.append(perm_o)
        in_maps.append({
            "v": np.ascontiguousarray(
                input_data[b, T_IN - 1, perm_i, 0], dtype=np.float32),
            "ic": np.ascontiguousarray(
                input_coords[b][perm_i], dtype=np.float32),
            "tc": np.ascontiguousarray(
                target_coords[b][perm_o], dtype=np.float32),
        })
    res = run_bass_kernel_spmd(nc, in_maps, list(range(B)), trace=trace)
    out = np.empty((B, S, N_OUT, T_OUT), dtype=np.float32)
    for b in range(B):
        out[b][:, perms_o[b], :] = res.results[b]["out"]
    return out, res


def kernel(input_data, input_coords, target_coords, n_samples):
    out, _ = _run(
        np.asarray(input_data),
        np.asarray(input_coords),
        np.asarray(target_coords),
        n_samples,
    )
    return out


# revision 21
# speedup vs baseline: 2.5539x; 2.5539x over previous
"""RBF/KNN interpolation kernel for Trainium2 (8 NeuronCores, data parallel).

Computes, per batch b:
    v        = input_data[b, -1, :, 0]                      (N_in,)
    w[o, i]  = exp(-||tc[o] - ic[i]||^2 / (2 * 0.1^2))      (N_out, N_in)
    interp   = (w @ v) / (w.sum(-1) + 1e-8)                 (N_out,)
    out[b]   = broadcast(interp) -> (n_samples, N_out, 4)

Sharding: batch B=8 across 8 cores (one batch per core).

v2 strategy on top of the v1 on-chip weight-matrix build:
  - Host sorts input points and target points by x per batch (a pure
    permutation; inverse-applied to the output). With sorted points,
    i-chunks (128 inputs) and o-subtiles (512 targets) cover narrow
    x-quantile windows, so (i-chunk, o-sub) pairs whose nominal windows
    are farther apart than D_WINDOW are statically pruned: their RBF
    weights are < exp(-50 * 0.3^2) and contribute ~0 to num/den.
  - The Exp over the kept logit tiles is split across engines: the
    Scalar engine runs true Exp; the Vector engine runs a Schraudolph
    fp16 bit-trick exp (y = A*logit + B -> int16 -> bitcast fp16), with
    the clamp/convert second pass placed on Vector or GpSimd by a
    compile-time greedy load balancer.
  - The PE stream is software-pipelined: the num/den matmul of tile t
    is emitted after the logits matmul of tile t+1 so the in-order PE
    never stalls waiting for an exp.
"""

from contextlib import ExitStack
from functools import lru_cache

import numpy as np

import concourse.bass as bass
import concourse.bacc as bacc
import concourse.tile as tile
from concourse import mybir
from concourse.bass_utils import run_bass_kernel_spmd

F32 = mybir.dt.float32
F16 = mybir.dt.float16
I16 = mybir.dt.int16
AF = mybir.ActivationFunctionType
ALU = mybir.AluOpType

# Problem sizes (hardcoded per spec)
B = 8
T_IN = 4
N_IN = 4096
V_IN = 3
N_OUT = 8192
S = 10
T_OUT = 4
GAMMA = 50.0  # 1 / (2 * LENGTH_SCALE^2), LENGTH_SCALE = 0.1
EPS = 1e-8
WSCALE_LOG = 6.93147180559945  # ln(2^10)
WSCALE = 1024.0

# pruning: drop (i-chunk, o-sub) pairs whose nominal x-quantile windows are
# farther apart than D_WINDOW (includes sort-quantile slack ~0.035)
D_WINDOW = 0.39
IC = N_IN // 128      # 32 i-chunks
OSUB_W = 512
OSUB = N_OUT // OSUB_W  # 16 o-subtiles

# Schraudolph fast-exp constants. Working in t-units (t = y/1024 = fp16
# exponent+15): t = SCH_SCALE*psum + (A_LN*bias + SCH_B)/1024; the stored
# bits are i16 = round(1024*t) clamped at 0 and the mantissa-linear error
# is corrected by w *= 1 + SCH_C*(s^2 - |s|), s = round(t) - t.
A_LN = 1024.0 * 1.4426950408889634           # 1024*log2(e)
SCH_SCALE = 2.0 * GAMMA * A_LN / 1024.0
SCH_B = 15.0                                  # fp16 exponent bias, t-units
EXP_SPLIT = False
SCH_C = 0.2355                                # quadratic mantissa correction coef
PIPE_DELAY = 5                                # tiles between logits emit and nd emit

# per-pass engine cost estimates (ns) for the compile-time balancer
def _t_scalar(cols):  # true exp on Activation engine
    return 0.833 * cols + 216.0
def _t_f32(cols):     # f32 pass on DVE (p1 psum, p2 conv, p3 magic, p4 s)
    return 1.0417 * cols + 90.0
def _t_fast(cols):    # f16 passes p5..p8, assumed 2x
    return 0.55 * cols + 80.0
def _t_gp(cols):      # a pass on GpSimd (eff ~0.6)
    return 1.39 * cols + 100.0


def _plan_tiles():
    """Static pruning plan. Returns (tiles, ic_lo, ic_hi) where tiles is the
    ordered list of (oc, ici, subs, c0, c1) and subs is the kept o-subs."""
    kept = {}
    for i in range(IC):
        for j in range(OSUB):
            ilo, ihi = i / IC, (i + 1) / IC
            olo, ohi = j / OSUB, (j + 1) / OSUB
            gap = max(olo - ihi, ilo - ohi, 0.0)
            if gap < D_WINDOW:
                kept.setdefault(j, []).append(i)
    ic_lo = {j: min(v) for j, v in kept.items()}
    ic_hi = {j: max(v) for j, v in kept.items()}
    tiles = []
    OC = N_OUT // 1024
    for oc in range(OC):
        ja, jb = 2 * oc, 2 * oc + 1
        lo = min(ic_lo[ja], ic_lo[jb])
        hi = max(ic_hi[ja], ic_hi[jb])
        for ici in range(lo, hi + 1):
            subs = [j for j in (ja, jb) if ic_lo[j] <= ici <= ic_hi[j]]
            if not subs:
                continue
            c0 = (min(subs) - ja) * OSUB_W
            c1 = (max(subs) + 1 - ja) * OSUB_W
            tiles.append((oc, ici, subs, c0, c1))
    return tiles, ic_lo, ic_hi


def _assign_engines(tiles):
    """Greedy balance across Scalar/DVE/GpSimd.

    Modes: 'S' scalar true exp; 'V' full Schraudolph+correction on DVE;
    'VG' DVE runs p1/p3/p4/p5, GpSimd runs p2/p6; 'VG2' GpSimd runs p2 only.
    """
    if not EXP_SPLIT:
        return ["S"] * len(tiles)
    load = {"S": 2500.0, "V": 14000.0, "G": 16000.0}
    out = []
    for (_, _, _, c0, c1) in tiles:
        cols = c1 - c0
        cand = []
        for mode in ("S", "V", "VG", "VG2"):
            trial = dict(load)
            if mode == "S":
                trial["S"] += _t_scalar(cols)
            elif mode == "V":
                trial["V"] += 4 * _t_f32(cols) + 4 * _t_fast(cols)
            elif mode == "VG":
                # GP takes p2 (conv) and p8 (final multiply)
                trial["V"] += 3 * _t_f32(cols) + 3 * _t_fast(cols)
                trial["G"] += 2 * _t_gp(cols)
            else:
                # GP takes p2 only
                trial["V"] += 3 * _t_f32(cols) + 4 * _t_fast(cols)
                trial["G"] += _t_gp(cols)
            cand.append((max(trial.values()), sum(trial.values()), mode, trial))
        cand.sort(key=lambda x: (x[0], x[1]))
        _, _, mode, trial = cand[0]
        load = trial
        out.append(mode)
    return out


def build_kernel(tc_ctx, v_h, ic_h, tc_h, out_h, n_in, n_out, s, F=1024):
    tcx = tc_ctx
    nc = tcx.nc
    OC = n_out // F    # o-chunks
    L = n_out // 128   # per-partition interp count in output layout
    CT = n_out // 128  # nat-layout columns (target side)

    tiles, ic_lo, ic_hi = _plan_tiles()
    modes = _assign_engines(tiles)
    # last tile index per oc (for finalize emission)
    last_of_oc = {}
    for idx, (oc, *_rest) in enumerate(tiles):
        last_of_oc[oc] = idx

    with ExitStack() as ctx:
        const_pool = ctx.enter_context(tcx.tile_pool(name="const", bufs=1))

        # ---- persistent tiles ----
        tc_aug = const_pool.tile([128, n_out], F16)  # rows t2h t2l txh txl txh tyh tyl tyh
        ic_aug = const_pool.tile([128, n_in], F16)   # rows 1   1   xh  xh  xl  yh  yh  yl
        bias_nat = const_pool.tile([128, IC], F32)
        ybias_nat = const_pool.tile([128, IC], F32)  # A_LN*bias + SCH_B
        vo_nat = const_pool.tile([128, 33 * IC], F16)  # [v, 0 x31, 1] per chunk
        ident = const_pool.tile([128, 128], F16)

        # ---- head: inputs, identity, splits ----
        head = ctx.enter_context(tcx.tile_pool(name="head", bufs=1))
        tcx_nat = head.tile([128, CT], F32)
        tcy_nat = head.tile([128, CT], F32)
        icx_nat = head.tile([128, IC], F32)
        icy_nat = head.tile([128, IC], F32)
        v_nat = head.tile([128, IC], F32)

        # PE clock warm-up (HAM un-throttles after ~3.4us of full-array work)
        warm_cm = tcx.tile_pool(name="warm_ps", bufs=2, space="PSUM")
        warm_ps = warm_cm.__enter__()
        wsrc = head.tile([128, 512], F16)
        nc.gpsimd.memset(wsrc[:, :].bitcast(mybir.dt.uint32), 0)

        tc_aug_u = tc_aug[:, :].bitcast(mybir.dt.uint32)
        ic_aug_u = ic_aug[:, :].bitcast(mybir.dt.uint32)
        nc.vector.memset(tc_aug_u[:, :tc_aug_u.shape[1] // 2], 0)
        nc.gpsimd.memset(tc_aug_u[:, tc_aug_u.shape[1] // 2:], 0)
        nc.vector.memset(ic_aug_u, 0)

        tc_r = tc_h[:].rearrange("(c p) d -> p c d", p=128)
        ic_r = ic_h[:].rearrange("(c p) d -> p c d", p=128)
        h = CT // 2
        nc.sync.dma_start(out=tcx_nat[:, 0:h].rearrange("p (c o) -> p c o", o=1),
                          in_=tc_r[:, 0:h, 0:1])
        nc.gpsimd.dma_start(out=tcx_nat[:, h:].rearrange("p (c o) -> p c o", o=1),
                            in_=tc_r[:, h:, 0:1])
        nc.scalar.dma_start(out=tcy_nat[:, 0:h].rearrange("p (c o) -> p c o", o=1),
                            in_=tc_r[:, 0:h, 1:2])
        nc.sync.dma_start(out=tcy_nat[:, h:].rearrange("p (c o) -> p c o", o=1),
                          in_=tc_r[:, h:, 1:2])
        nc.gpsimd.dma_start(out=icx_nat.rearrange("p (c o) -> p c o", o=1),
                            in_=ic_r[:, :, 0:1])
        nc.scalar.dma_start(out=icy_nat.rearrange("p (c o) -> p c o", o=1),
                            in_=ic_r[:, :, 1:2])
        nc.sync.dma_start(out=v_nat[:, :],
                          in_=v_h[:].rearrange("(c p) -> p c", p=128))

        # identity for PE transposes
        jj = head.tile([128, 128], F32)
        kk = head.tile([128, 1], F32)
        nc.gpsimd.iota(jj[:, :], [[1, 128]], base=0, channel_multiplier=0,
                       allow_small_or_imprecise_dtypes=True)
        nc.gpsimd.iota(kk[:, :], [[0, 1]], base=0, channel_multiplier=1,
                       allow_small_or_imprecise_dtypes=True)
        nc.gpsimd.tensor_scalar(ident[:, :], jj[:, :], kk[:, 0:1], None,
                                op0=ALU.is_equal)

        # --- target-side nat computes ---
        sqx_t = head.tile([128, CT], F32)
        t2s = head.tile([128, CT], F32)
        nc.gpsimd.tensor_mul(sqx_t[:, :], tcx_nat[:, :], tcx_nat[:, :])
        nc.vector.scalar_tensor_tensor(t2s[:, :], tcy_nat[:, :], 1.0,
                                       tcy_nat[:, :], op0=ALU.bypass,
                                       op1=ALU.mult)
        nc.vector.tensor_add(t2s[:, :], t2s[:, :], sqx_t[:, :])

        t2h_nat = head.tile([128, CT], F16)
        t2l_nat = head.tile([128, CT], F16)
        nc.vector.tensor_scalar_mul(t2h_nat[:, :], t2s[:, :], -0.5)
        nc.vector.scalar_tensor_tensor(t2l_nat[:, :], t2s[:, :], -0.5,
                                       t2h_nat[:, :], op0=ALU.mult,
                                       op1=ALU.subtract)
        txh_nat = head.tile([128, CT], F16)
        txl_nat = head.tile([128, CT], F16)
        tyh_nat = head.tile([128, CT], F16)
        tyl_nat = head.tile([128, CT], F16)
        nc.gpsimd.tensor_copy(txh_nat[:, :], tcx_nat[:, :])
        nc.vector.tensor_sub(txl_nat[:, :], tcx_nat[:, :], txh_nat[:, :])
        nc.gpsimd.tensor_copy(tyh_nat[:, :], tcy_nat[:, :])
        nc.vector.tensor_sub(tyl_nat[:, :], tcy_nat[:, :], tyh_nat[:, :])

        # --- input-side nat computes ---
        sqx_i = head.tile([128, IC], F32)
        i2s = head.tile([128, IC], F32)
        nc.gpsimd.tensor_mul(sqx_i[:, :], icx_nat[:, :], icx_nat[:, :])
        nc.vector.scalar_tensor_tensor(i2s[:, :], icy_nat[:, :], 1.0,
                                       icy_nat[:, :], op0=ALU.bypass,
                                       op1=ALU.mult)
        nc.vector.tensor_add(i2s[:, :], i2s[:, :], sqx_i[:, :])
        # bias = -50 * i2 + ln(2^10);  ybias = A_LN * bias + SCH_B
        nc.vector.tensor_scalar(bias_nat[:, :], i2s[:, :], -GAMMA, WSCALE_LOG,
                                op0=ALU.mult, op1=ALU.add)
        nc.vector.tensor_scalar(ybias_nat[:, :], bias_nat[:, :], A_LN / 1024.0,
                                SCH_B, op0=ALU.mult, op1=ALU.add)

        xh_nat = head.tile([128, IC], F16)
        xl_nat = head.tile([128, IC], F16)
        yh_nat = head.tile([128, IC], F16)
        yl_nat = head.tile([128, IC], F16)
        nc.gpsimd.tensor_copy(xh_nat[:, :], icx_nat[:, :])
        nc.vector.tensor_sub(xl_nat[:, :], icx_nat[:, :], xh_nat[:, :])
        nc.gpsimd.tensor_copy(yh_nat[:, :], icy_nat[:, :])
        nc.vector.tensor_sub(yl_nat[:, :], icy_nat[:, :], yh_nat[:, :])

        # vo_nat: col 33c = v (fp16), col 33c+32 = 1.0, rest 0
        nc.gpsimd.memset(vo_nat[:, :].bitcast(mybir.dt.uint16), 0)
        vo3 = vo_nat.rearrange("p (c w) -> p c w", w=33)
        nc.vector.tensor_copy(vo3[:, :, 0], v_nat[:, :])
        nc.vector.memset(vo3[:, :, 32], 1.0)

        # --- nat -> row layout via PE transpose + copy + DMA ---
        with tcx.tile_pool(name="tps", bufs=2, space="PSUM") as tp_pool, \
             tcx.tile_pool(name="tsb", bufs=2) as tsb_pool:

            tp_count = [0]

            dma_engs = [nc.sync, nc.scalar, nc.gpsimd]

            def to_rows(nat, ncols, aug, rows):
                ps = tp_pool.tile([128, 128], F16, tag="ps")
                sb = tsb_pool.tile([128, 128], F16, tag="sb")
                nc.tensor.transpose(ps[:ncols, :], nat[:, :], ident[:, :])
                if tp_count[0] % 2 == 0:
                    nc.vector.tensor_copy(sb[:ncols, :], ps[:ncols, :])
                else:
                    nc.scalar.copy(sb[:ncols, :], ps[:ncols, :])
                for r in rows:
                    dma_engs[tp_count[0] % 3].dma_start(
                        out=aug[r:r + 1, :].rearrange("r (c p) -> r c p", p=128),
                        in_=sb[:ncols, :],
                    )
                    tp_count[0] += 1

            to_rows(t2h_nat, CT, tc_aug, [0])
            to_rows(t2l_nat, CT, tc_aug, [1])
            to_rows(txh_nat, CT, tc_aug, [2, 4])
            to_rows(txl_nat, CT, tc_aug, [3])
            to_rows(tyh_nat, CT, tc_aug, [5, 7])
            to_rows(tyl_nat, CT, tc_aug, [6])

            nc.vector.memset(ic_aug[0:2, :].bitcast(mybir.dt.uint32), 0x3C003C00)
            to_rows(xh_nat, IC, ic_aug, [2, 3])
            to_rows(xl_nat, IC, ic_aug, [4])
            to_rows(yh_nat, IC, ic_aug, [5, 6])
            to_rows(yl_nat, IC, ic_aug, [7])
            # HAM warm-up bridge: ~5us of sustained full-array matmuls right
            # before the main loop un-throttles the PE clock (1.2 -> 2.4 GHz)
            for _ in range(24):
                wps = warm_ps.tile([128, 512], F32, tag="warm")
                nc.tensor.matmul(wps[:, :], wsrc[:, 0:128], wsrc[:, :],
                                 start=True, stop=True)

        warm_cm.__exit__(None, None, None)

        # ---- main loop (software-pipelined) ----
        PG2 = F // L  # output partitions finalized per o-chunk

        with (
            tcx.tile_pool(name="psum_l", bufs=3, space="PSUM") as pl_pool,
            tcx.tile_pool(name="psum_nd", bufs=1, space="PSUM") as nd_pool,
            tcx.tile_pool(name="w", bufs=8) as w_pool,
            tcx.tile_pool(name="y", bufs=4) as y_pool,
            tcx.tile_pool(name="sch", bufs=3) as sch_pool,
            tcx.tile_pool(name="grp", bufs=2) as grp_pool,
        ):
            # one persistent psum tile; even oc accumulates at partitions
            # 0..32, odd oc at 64..96 (num row +0, den row +32)
            nd = nd_pool.tile([128, F], F32, tag="nd", name="nd")
            pending = []  # (tile_idx, oc, ici, subs, w_tile)

            def emit_front(idx):
                oc, ici, subs, c0, c1 = tiles[idx]
                mode = modes[idx]
                pl = pl_pool.tile([128, F], F32, tag="pl")
                for j in subs:
                    s0 = (j - 2 * oc) * OSUB_W
                    nc.tensor.matmul(
                        pl[:, s0:s0 + OSUB_W],
                        ic_aug[:, ici * 128:(ici + 1) * 128],
                        tc_aug[:, j * OSUB_W:(j + 1) * OSUB_W],
                        start=True, stop=True,
                    )
                w = w_pool.tile([128, F], F16, tag="w")
                if mode == "S":
                    nc.scalar.activation(
                        w[:, c0:c1], pl[:, c0:c1], AF.Exp,
                        bias=bias_nat[:, ici:ici + 1], scale=2.0 * GAMMA,
                    )
                else:
                    # p1: t = fp16-exponent+15 of the weight, in t-units
                    t = y_pool.tile([128, F], F32, tag="t")
                    nc.vector.tensor_scalar(
                        t[:, c0:c1], pl[:, c0:c1],
                        SCH_SCALE, ybias_nat[:, ici:ici + 1],
                        op0=ALU.mult, op1=ALU.add,
                    )
                    # p2: stored bits = round(1024*t) clamped at 0
                    wi = w[:, c0:c1].bitcast(I16)
                    eng2 = nc.gpsimd if mode in ("VG", "VG2") else nc.vector
                    eng2.tensor_scalar(wi, t[:, c0:c1], 1024.0, 0.0,
                                       op0=ALU.mult, op1=ALU.max)
                    # p3/p4: s = round(t) - t via the fp32 2^23 magic trick
                    z = y_pool.tile([128, F], F32, tag="z")
                    ss = sch_pool.tile([128, F], F16, tag="ss")
                    u = sch_pool.tile([128, F], F16, tag="u")
                    hh = sch_pool.tile([128, F], F16, tag="hh")
                    nc.vector.tensor_scalar(
                        z[:, c0:c1], t[:, c0:c1], 2.0 ** 23, None, op0=ALU.add)
                    nc.vector.scalar_tensor_tensor(
                        ss[:, c0:c1], z[:, c0:c1], -2.0 ** 23, t[:, c0:c1],
                        op0=ALU.add, op1=ALU.subtract,
                    )
                    # p5..p8: w *= 1 + SCH_C*(|s|^2 - |s|)  (== f^2 - f)
                    nc.vector.scalar_tensor_tensor(
                        u[:, c0:c1], ss[:, c0:c1], -1.0, ss[:, c0:c1],
                        op0=ALU.mult, op1=ALU.max,
                    )
                    nc.vector.scalar_tensor_tensor(
                        u[:, c0:c1], u[:, c0:c1], -1.0, u[:, c0:c1],
                        op0=ALU.add, op1=ALU.mult,
                    )
                    nc.vector.tensor_scalar(
                        hh[:, c0:c1], u[:, c0:c1], SCH_C, 1.0,
                        op0=ALU.mult, op1=ALU.add,
                    )
                    eng8 = nc.gpsimd if mode == "VG" else nc.vector
                    eng8.tensor_mul(w[:, c0:c1], w[:, c0:c1], hh[:, c0:c1])
                pending.append((idx, oc, ici, subs, w))

            def emit_back():
                idx, oc, ici, subs, w = pending.pop(0)
                poff = 64 * (oc % 2)
                for j in subs:
                    s0 = (j - 2 * oc) * OSUB_W
                    nc.tensor.matmul(
                        nd[poff:poff + 33, s0:s0 + OSUB_W],
                        vo_nat[:, 33 * ici:33 * ici + 33],
                        w[:, s0:s0 + OSUB_W],
                        start=(ici == ic_lo[j]), stop=(ici == ic_hi[j]),
                        skip_group_check=True,
                    )
                if idx == last_of_oc[oc]:
                    finalize(oc)

            def finalize(oc):
                poff = 64 * (oc % 2)
                nd_rows = grp_pool.tile([33, F], F32, tag="ndrows")
                nc.vector.tensor_copy(nd_rows[0:33, :], nd[poff:poff + 33, :])
                gnum = grp_pool.tile([PG2, L], F32, tag="gnum")
                gden = grp_pool.tile([PG2, L], F32, tag="gden")
                grep = grp_pool.tile([PG2, 4 * L], F32, tag="grep")
                nc.sync.dma_start(
                    out=gnum[:, :],
                    in_=nd_rows[0:1, :].rearrange("r (p k) -> r p k", k=L),
                )
                nc.gpsimd.dma_start(
                    out=gden[:, :],
                    in_=nd_rows[32:33, :].rearrange("r (p k) -> r p k", k=L),
                )
                nc.vector.tensor_scalar_add(gden[:, :], gden[:, :], EPS * WSCALE)
                nc.vector.reciprocal(gden[:, :], gden[:, :])
                nc.vector.tensor_mul(gnum[:, :], gnum[:, :], gden[:, :])
                grep3 = grep.rearrange("p (k t) -> p k t", t=4)
                for t in range(4):
                    nc.vector.tensor_copy(grep3[:, :, t], gnum[:, :])
                engs = [nc.sync, nc.gpsimd]
                for si in range(s):
                    engs[si % len(engs)].dma_start(
                        out=out_h[:][si].rearrange("o t -> (o t)").rearrange(
                            "(p j) -> p j", p=n_out * 4 // (4 * L))[
                                oc * PG2:(oc + 1) * PG2, :],
                        in_=grep[:, :],
                    )

            # software pipeline: nd matmul of tile t lands PIPE_DELAY after
            # its logits matmul, so the in-order PE never waits on an exp
            for idx in range(len(tiles)):
                emit_front(idx)
                if len(pending) > PIPE_DELAY:
                    emit_back()
            while pending:
                emit_back()


@lru_cache(maxsize=2)
def build_nc(n_in=N_IN, n_out=N_OUT, s=S, F=1024):
    nc = bacc.Bacc("TRN2", target_bir_lowering=False, debug=False)
    v_h = nc.dram_tensor("v", [n_in], F32, kind="ExternalInput")
    ic_h = nc.dram_tensor("ic", [n_in, 2], F32, kind="ExternalInput")
    tc_h = nc.dram_tensor("tc", [n_out, 2], F32, kind="ExternalInput")
    out_h = nc.dram_tensor("out", [s, n_out, T_OUT], F32, kind="ExternalOutput")
    with tile.TileContext(nc) as tcx:
        build_kernel(tcx, v_h, ic_h, tc_h, out_h, n_in, n_out, s, F=F)
    nc.compile()
    return nc


def _run(input_data, input_coords, target_coords, n_samples, trace=False):
    n_samples = int(n_samples)
    assert n_samples == S, f"kernel compiled for n_samples={S}, got {n_samples}"
    assert input_data.shape == (B, T_IN, N_IN, V_IN)
    nc = build_nc()
    perms_o = []
    in_maps = []
    for b in range(B):
        perm_i = np.argsort(input_coords[b, :, 0], kind="stable")
        perm_o = np.argsort(target_coords[b, :, 0], kind="stable")
        perms_o.append(perm_o)
        in_maps.append({
            "v": np.ascontiguousarray(
                input_data[b, T_IN - 1, perm_i, 0], dtype=np.float32),
            "ic": np.ascontiguousarray(
                input_coords[b][perm_i], dtype=np.float32),
            "tc": np.ascontiguousarray(
                target_coords[b][perm_o], dtype=np.float32),
        })
    res = run_bass_kernel_spmd(nc, in_maps, list(range(B)), trace=trace)
    out = np.empty((B, S, N_OUT, T_OUT), dtype=np.float32)
    for b in range(B):
        out[b][:, perms_o[b], :] = res.results[b]["out"]
    return out, res


def kernel(input_data, input_coords, target_coords, n_samples):
    out, _ = _run(
        np.asarray(input_data),
        np.asarray(input_coords),
        np.asarray(target_coords),
        n_samples,
    )
    return out
